# revision 52
# baseline (speedup 1.0000x reference)
"""Multi-head self-attention (RoPE, causal) on 8 Trainium2 NeuronCores.

Sharding: tensor-parallel over heads - 2 of 16 heads per core. Each core
computes its heads' Q/K/V projections (single projection per tensor; RoPE is
applied with DVE muls plus a stream_shuffle partition swap), causal
flash-style attention in a transposed [dim, seq] layout, and a partial output
projection (bf16) against its row-slice of w_o. Partial outputs are written
as fp16 and summed on the host.

Projection work for step st+1 is interleaved into the attention kc-loop of
step st (emission-order fillers) so the scalar engine's softmax exp - the
binding resource - starts early and the PE never idles long enough to drop
out of the HAM fast clock state.
"""

import sys

sys.path.insert(0, "/opt/trn_rl_repo")
sys.path.insert(0, "/root/problem")

import numpy as np
import ml_dtypes

import concourse.bass as bass
import concourse.tile as tile

# ---------------------------------------------------------------------------
# Toolchain fixes (inlined, self-contained): walrus on this stack allows only
# one sync-wait per instruction; the Tile tail drain carries many.  Also the
# image's antenv lacks the NTFF profile hook.
# ---------------------------------------------------------------------------
from concourse.vector_clock import ScopedClock

MAXW = 1


def _patched_drain_and_barrier(self, tick_clock, wait_clock):
    nc = self.nc
    drain_inst = nc.sync.drain()
    wait_clock.add_sem_waits(
        drain_inst.ins, ScopedClock({None: tick_clock.global_clock})
    )
    si = drain_inst.ins.sync_info
    waits = list(si.on_wait or []) if si is not None else []
    if len(waits) > MAXW:
        si.on_wait = waits[:MAXW]
        rest = waits[MAXW:]
        while rest:
            chunk, rest = rest[:MAXW], rest[MAXW:]
            nop = nc.sync.nop(nofuse=True)
            nsi = nop.ins.sync_info
            if nsi is None:
                import bass_rust

                nop.ins.sync_info = bass_rust.SyncInfo(on_wait=chunk, on_update=[])
            else:
                nsi.on_wait = list(nsi.on_wait or []) + chunk

    nc.all_engine_barrier()
    assert self.sems is not None
    popped = nc._tile_sem_poison_stack.pop()
    assert popped is self._sem_poison
    nc.clear_and_free_semaphores(list(self.sems.allocated().values()))
    nc.all_engine_barrier()


def apply():
    tile.TileContext._drain_and_barrier = _patched_drain_and_barrier
    _install_ntff_hook_shim()
    _install_compile_hook()


def _split_waits_json(bir_json: bytes) -> bytes:
    """Walrus on this toolchain allows at most one sync-wait per instruction.
    Insert a same-engine NoOp carrying each excess wait immediately before any
    multi-wait instruction (engine blocks at the NoOp instead - identical
    semantics, order preserved)."""
    import json as _json

    d = _json.loads(bir_json)
    n_split = 0
    for fn in d.get("functions", []):
        for bb in fn.get("blocks", []):
            insts = bb.get("instructions", [])
            out = []
            for inst in insts:
                si = inst.get("sync_info")
                waits = (si or {}).get("on_wait") or []
                if len(waits) > 1:
                    ge = [w for w in waits if w.get("wait_mode") == "sem-ge-imm"]
                    other = [w for w in waits if w.get("wait_mode") != "sem-ge-imm"]
                    # keep one wait on the instruction (prefer a non-ge if present)
                    if other:
                        keep = other
                        move = ge
                    else:
                        keep = ge[-1:]
                        move = ge[:-1]
                    if len(keep) <= 1 and move:
                        for i, w in enumerate(move):
                            out.append(
                                {
                                    "debug": inst.get("debug", 0),
                                    "engine": inst["engine"],
                                    "ins": [],
                                    "outs": [],
                                    "name": f"{inst['name']}-ws{i}",
                                    "opcode": "NoOp",
                                    "sync_info": {"on_update": [], "on_wait": [w]},
                                }
                            )
                            n_split += 1
                        si["on_wait"] = keep
                out.append(inst)
            bb["instructions"] = out
    if n_split:
        print(f"tilefix: split {n_split} excess waits onto NoOps")
    return _json.dumps(d).encode()


def _install_compile_hook():
    import concourse.bass_utils as bu
    import concourse.bass2jax as b2j

    if getattr(bu, "_tilefix_wrapped", False):
        return
    orig = bu.compile_bir_kernel

    def wrapped(bir_json, tmpdir, neff_name="file.neff"):
        return orig(_split_waits_json(bir_json), tmpdir, neff_name)

    bu.compile_bir_kernel = wrapped
    b2j.compile_bir_kernel = wrapped
    bu._tilefix_wrapped = True


def _install_ntff_hook_shim():
    """The image's antenv package lacks axon_hooks; provide a stand-in module
    exposing the ctypes-based NTFF profile hook against /opt/axon/libaxon_pjrt.so
    so run_bass_kernel_spmd(trace=True) works."""
    import sys as _sys
    import types

    if "antenv.axon_hooks" in _sys.modules:
        return
    mod = types.ModuleType("antenv.axon_hooks")
    _state = {"hook": None}

    so_path = "/opt/axon/libaxon_pjrt.so"
    try:
        import importlib.util

        spec = importlib.util.spec_from_file_location(
            "trn_agent_boot.trn_boot", "/root/.axon_site/trn_agent_boot/trn_boot.py"
        )
        # trn_boot is already importable as a package in the axon site; reuse it.
        import trn_agent_boot.trn_boot as _tb  # type: ignore

        _state["hook"] = _tb._ntff_profile_via_ctypes(so_path)
    except Exception:
        _state["hook"] = None

    def get_axon_ntff_profile_hook():
        return _state["hook"]

    def set_axon_ntff_profile_hook(h):
        _state["hook"] = h

    mod.get_axon_ntff_profile_hook = get_axon_ntff_profile_hook
    mod.set_axon_ntff_profile_hook = set_axon_ntff_profile_hook
    _sys.modules["antenv.axon_hooks"] = mod

apply()

from concourse import mybir
from concourse.bass_utils import run_bass_kernel_spmd
from concourse.masks import make_identity

F32 = mybir.dt.float32
F32R = mybir.dt.float32r
BF16 = mybir.dt.bfloat16
FP16 = mybir.dt.float16
EXP = mybir.ActivationFunctionType.Exp
LN = mybir.ActivationFunctionType.Ln
COPY = mybir.ActivationFunctionType.Copy

S = 4096          # sequence length
D = 1024          # model dim
NH = 16           # heads
HD = 64           # head dim
NCORES = 8
HPC = NH // NCORES  # heads per core = 2
QT = 512          # qpos tile (free dim of S^T / PV matmuls)
KC = 128          # kpos chunk (partition dim of S^T tiles)
NQT = S // QT     # 8
NKC = S // KC     # 32

# stream_shuffle operates within 32-partition groups: mask[i] = source lane
# (0..31) for output lane i of every group.  Swap 16-lane halves (p ^= 16).
SWAPMASK = list(range(16, 32)) + list(range(0, 16))

_CACHE = {}


def _build_nc():
    nc = bass.Bass("TRN2")

    xT_d = nc.dram_tensor("xT", [D, S], BF16, kind="ExternalInput")
    wq_d = nc.dram_tensor("wq", [D, 128], BF16, kind="ExternalInput")
    wk_d = nc.dram_tensor("wk", [D, 128], BF16, kind="ExternalInput")
    wv_d = nc.dram_tensor("wv", [D, 128], BF16, kind="ExternalInput")
    wo_d = nc.dram_tensor("wo", [128, D], BF16, kind="ExternalInput")
    cos_d = nc.dram_tensor("cosP", [128, S], BF16, kind="ExternalInput")
    sin_d = nc.dram_tensor("sinPs", [128, S], BF16, kind="ExternalInput")
    out_d = nc.dram_tensor("out", [S, D], FP16, kind="ExternalOutput")

    with tile.TileContext(nc) as tc:
        with (
            tc.tile_pool(name="const", bufs=1) as cpool,
            tc.tile_pool(name="big", bufs=1) as big,
            tc.tile_pool(name="xp", bufs=3) as xp,
            tc.tile_pool(name="pt", bufs=10) as ptp,
            tc.tile_pool(name="ob", bufs=2) as obp,
            tc.tile_pool(name="sm", bufs=4) as smp,
            tc.tile_pool(name="ps", bufs=2, space="PSUM") as ps,
            tc.tile_pool(name="pss", bufs=2, space="PSUM") as pssp,
            tc.tile_pool(name="po", bufs=2, space="PSUM") as po,
        ):
            # ---- constants -------------------------------------------------
            identb = cpool.tile([128, 128], BF16)
            make_identity(nc, identb[:])
            cosP = cpool.tile([128, S], BF16)
            sinPs = cpool.tile([128, S], BF16)
            wo_sb = cpool.tile([128, D], BF16)

            # wq first so the first projection can start as early as possible;
            # the first x tile's DMAs are issued right after (see below)
            w_sb = {}
            for name in ("q", "k", "v"):
                w_sb[name] = cpool.tile([128, 8, 128], BF16, name=f"w_{name}")
            w_dram = {"q": wq_d, "k": wk_d, "v": wv_d}

            def w_dma(name):
                nc.sync.dma_start(
                    w_sb[name][:],
                    w_dram[name].ap().rearrange("(kc p) m -> p kc m", p=128),
                )

            # ones1 / vaug ones-columns built from identb (no DMA dependency)
            ones1 = cpool.tile([1, 64], FP16)
            nc.scalar.activation(ones1[:], identb[0:1, 0:64], COPY, bias=1.0, scale=0.0)

            # ---- persistent activations -----------------------------------
            qT = big.tile([128, S], BF16)   # rope'd q, [2*64 dims, seq]
            kT = big.tile([128, S], BF16)
            attnT = big.tile([128, S], BF16)  # normalized attn out, [dims, seq]
            vaug = [big.tile([128, NKC * 65], BF16, name=f"vaug{h}") for h in range(HPC)]
            for h in range(HPC):
                va = vaug[h].rearrange("p (kc m) -> p kc m", m=65)
                nc.scalar.activation(
                    va[:, :, 64:65], identb[:, 0:NKC], COPY, bias=1.0, scale=0.0
                )

            def warm_pe():
                # keep the PE busy on junk matmuls while the first DMAs land
                # so the HAM clock gate is already at 8/8 for the real work
                junk = pssp.tile([128, QT], F32, tag="pss")
                for r in range(16):
                    nc.tensor.matmul(junk[:, 0:128], identb[:], identb[:],
                                     start=True, stop=True)

            # ---------------------------------------------------------------
            # emission pieces
            # ---------------------------------------------------------------
            xblks = {}

            def dma_piece(st):
                sl = slice(st * QT, (st + 1) * QT)
                xblk = xp.tile([128, 8, QT], BF16, tag="xblk")
                xblks[st] = xblk
                if st == 0:
                    # chunked so the first matmuls can start early
                    x4 = xT_d.ap().rearrange("(kc p) m -> p kc m", p=128)
                    for kc in range(0, 8, 2):
                        nc.sync.dma_start(
                            xblk[:, kc:kc + 2, :], x4[:, kc:kc + 2, sl]
                        )
                else:
                    # issued a full step ahead - one DMA saves issue overhead
                    nc.sync.dma_start(
                        xblk[:],
                        xT_d.ap().rearrange("(kc p) m -> p kc m", p=128)[:, :, sl],
                    )
                # stream this step's slice of the rope tables
                nc.sync.dma_start(cosP[:, sl], cos_d.ap()[:, sl])
                nc.sync.dma_start(sinPs[:, sl], sin_d.ap()[:, sl])

            def proj(st, wname):
                xblk = xblks[st]
                psum = ps.tile([128, QT], F32, tag="ps")
                for kc in range(8):
                    nc.tensor.matmul(
                        psum[:], w_sb[wname][:, kc, :], xblk[:, kc, :],
                        start=(kc == 0), stop=(kc == 7),
                    )
                return psum

            def qk_piece(st, wname, dst):
                sl = slice(st * QT, (st + 1) * QT)
                pa = proj(st, wname)
                t_sin = smp.tile([128, QT], BF16, tag="sin")
                nc.vector.tensor_mul(t_sin[:], pa[:], sinPs[:, sl])
                nc.vector.tensor_mul(dst[:, sl], pa[:], cosP[:, sl])
                t_shuf = smp.tile([128, QT], BF16, tag="shuf")
                nc.vector.stream_shuffle(t_shuf[:], t_sin[:], SWAPMASK)
                nc.vector.tensor_add(dst[:, sl], dst[:, sl], t_shuf[:])

            def v_piece(st):
                # compute v^T [kpos, dims] directly: stationary = x chunk,
                # moving = wv chunk (no PE transposes, no vtmp staging)
                xblk = xblks[st]
                pst_f = ps.tile([128, QT], F32, tag="ps")
                pst = pst_f[:].rearrange("p (z m) -> p z m", m=128)
                for z in range(4):
                    for dch in range(8):
                        nc.tensor.matmul(
                            pst[:, z, :], xblk[:, dch, z * 128:(z + 1) * 128],
                            w_sb["v"][:, dch, :],
                            start=(dch == 0), stop=(dch == 7),
                        )
                for h in range(HPC):
                    va = vaug[h].rearrange("p (kc m) -> p kc m", m=65)
                    nc.vector.tensor_copy(
                        va[:, st * 4:(st + 1) * 4, 0:64],
                        pst[:, :, h * 64:(h + 1) * 64],
                    )

            def wo_piece():
                nc.sync.dma_start(wo_sb[:], wo_d.ap())

            # ---- per-qt attention state ------------------------------------
            rcps = {}

            def fin_norm(qt):
                """normalize a finished qt into attnT (releases its PV psums)"""
                qsl = slice(qt * QT, (qt + 1) * QT)
                po_h = po_tiles[qt]
                rbs = []
                for h in range(HPC):
                    pb_ps = ps.tile([128, QT], F32, tag="ps", name=f"pb{qt}_{h}")
                    nc.tensor.matmul(
                        pb_ps[0:64, :], ones1[:], rcps[qt][h][:],
                        start=True, stop=True,
                    )
                    rb = smp.tile([64, QT], F32, tag="rb", name=f"rb{qt}_{h}")
                    nc.vector.tensor_copy(rb[:], pb_ps[0:64, :])
                    rbs.append(rb)
                for h in range(HPC):
                    hsl = slice(h * 64, (h + 1) * 64)
                    nc.vector.tensor_mul(
                        attnT[hsl, qsl], po_h[h][0:64, :], rbs[h][:]
                    )

            def fin_out(qt, split_dma=False):
                """output projection for a normalized qt (deferrable)"""
                osb = obp.tile([128, 4, 2, QT], FP16, tag="ob")
                for z in range(4):
                    csl = slice(qt * QT + z * 128, qt * QT + (z + 1) * 128)
                    for ncol in range(2):
                        osl = slice(ncol * 512, (ncol + 1) * 512)
                        ps_o = ps.tile([128, QT], F32, tag="ps")
                        nc.tensor.matmul(
                            ps_o[:], attnT[:, csl], wo_sb[:, osl],
                            start=True, stop=True,
                        )
                        nc.vector.tensor_copy(osb[:, z, ncol, :], ps_o[:])
                    if split_dma:
                        csl = slice(qt * QT + z * 128, qt * QT + (z + 1) * 128)
                        nc.sync.dma_start(
                            out_d.ap()[csl, :].rearrange("p (n m) -> p n m", m=512),
                            osb[:, z],
                        )
                if not split_dma:
                    nc.sync.dma_start(
                        out_d.ap()[qt * QT:(qt + 1) * QT, :].rearrange(
                            "(z p) (n m) -> p z n m", p=128, m=512
                        ),
                        osb[:],
                    )

            po_tiles = {}

            # global deferred-work queue: (due_qt, fn).  due_qt = qt at whose
            # attn_core start the piece MUST have been emitted (deps).
            pieces = []

            def pump_urgent(qt):
                for i, (due, _) in enumerate(pieces):
                    if due <= qt + 1:
                        pieces.pop(i)[1]()
                        return True
                return False

            def pump_head():
                if pieces:
                    pieces.pop(0)[1]()

            def drain_due(qt):
                i = 0
                while i < len(pieces):
                    if pieces[i][0] <= qt:
                        pieces.pop(i)[1]()
                    else:
                        i += 1

            def attn_core(qt):
                """scores + exp + PV for query tile qt, with deferred-piece
                injection.

                PV lags scores by 2 kc so fin_norm(qt-1) can be emitted after
                the first two score groups (hiding its PE stall), and so the
                pss ring never blocks the PE head-of-line.
                """
                n_kc = 4 * qt + 4
                qsl0 = qt * QT
                pts = {}

                # anything due by now must be emitted before the kc loop
                drain_due(qt)

                LAG = 8
                for kc in range(n_kc):
                    ksl = slice(kc * 128, (kc + 1) * 128)
                    joff = max(0, (kc - 4 * qt)) * 128
                    if kc == 4 and qt >= 1:
                        fin_norm(qt - 1)
                        pieces.append((10**9, lambda q=qt - 1: fin_out(q)))
                    if kc == 4 or (n_kc <= 4 and kc == 2):
                        po_tiles[qt] = [
                            po.tile([128, QT], F32, tag="po", name=f"po{qt}_{h}")
                            for h in range(HPC)
                        ]
                    ps_s = pssp.tile([128, 2 * QT], F32, tag="pss")
                    for h in range(HPC):
                        hsl = slice(h * 64, (h + 1) * 64)
                        nc.tensor.matmul(
                            ps_s[:, h * QT + joff:(h + 1) * QT],
                            kT[hsl, ksl],
                            qT[hsl, qsl0 + joff:qsl0 + QT],
                            start=True, stop=True,
                        )
                    pt_t = ptp.tile([128, 2 * QT], BF16, tag="pt")
                    pts[kc] = pt_t
                    ps3 = ps_s[:].rearrange("p (h q) -> p h q", h=HPC)
                    pt3 = pt_t[:].rearrange("p (h q) -> p h q", h=HPC)
                    nc.scalar.activation(
                        pt3[:, :, joff:QT], ps3[:, :, joff:QT], EXP, scale=0.125
                    )
                    if kc >= 4 * qt:
                        j = kc - 4 * qt
                        for h in range(HPC):
                            nc.gpsimd.affine_select(
                                out=pt_t[:, h * QT:(h + 1) * QT],
                                in_=pt_t[:, h * QT:(h + 1) * QT],
                                compare_op=mybir.AluOpType.is_ge,
                                fill=0.0, base=-(j * 128),
                                pattern=[[1, QT]], channel_multiplier=-1,
                            )
                    if kc >= LAG:
                        pv_step(qt, kc - LAG, pts, n_kc)
                        del pts[kc - LAG]
                    # proj pieces (due next qt) always flow; deferrable
                    # fin_out pieces are held for the late, PE-starved kcs
                    if kc % 3 == 2:
                        if not pump_urgent(qt):
                            # drip deferred fin_outs through the late qts,
                            # keeping one for the final reciprocal gap
                            if (qt == 6 and kc in (8, 20)) or \
                               (qt == 7 and kc in (8, 17, 23, 29)):
                                pump_head()
                # tail: finish each head's PV then immediately start its
                # reciprocal chain on the scalar engine (exp(-ln(d)))
                rcps[qt] = []
                for h in range(HPC):
                    for kc in sorted(pts):
                        pv_one(qt, kc, pts, n_kc, h)
                    tln = smp.tile([1, QT], F32, tag="tln", name=f"tln{qt}_{h}")
                    nc.scalar.activation(tln[:], po_tiles[qt][h][64:65, :], LN)
                    rcp = smp.tile([1, QT], FP16, tag="rcp", name=f"rcp{qt}_{h}")
                    nc.scalar.activation(rcp[:], tln[:], EXP, scale=-1.0)
                    rcps[qt].append(rcp)

            def pv_one(qt, kc, pts, n_kc, h):
                joff = max(0, (kc - 4 * qt)) * 128
                pt_t = pts[kc]
                po_h = po_tiles[qt]
                va = vaug[h].rearrange("p (kc m) -> p kc m", m=65)
                nc.tensor.matmul(
                    po_h[h][0:65, joff:QT], va[:, kc, :],
                    pt_t[:, h * QT + joff:(h + 1) * QT],
                    start=(kc == 0), stop=(kc == n_kc - 1),
                )

            def pv_step(qt, kc, pts, n_kc):
                for h in range(HPC):
                    pv_one(qt, kc, pts, n_kc, h)

            # ---------------------------------------------------------------
            # main schedule
            # ---------------------------------------------------------------
            # step 0 projections run inline (nothing to overlap them with);
            # wq and the first x tile stream first so the PE starts early
            w_dma("q")
            dma_piece(0)
            w_dma("k")
            w_dma("v")
            qk_piece(0, "q", qT)
            qk_piece(0, "k", kT)
            v_piece(0)

            for qt in range(NQT):
                if qt + 1 < NQT:
                    st = qt + 1
                    dma_piece(st)  # issue input DMAs early
                    if qt == 0:
                        pieces.append((1, wo_piece))
                    pieces.append((st, lambda st=st: qk_piece(st, "q", qT)))
                    pieces.append((st, lambda st=st: qk_piece(st, "k", kT)))
                    pieces.append((st, lambda st=st: v_piece(st)))
                attn_core(qt)
            while pieces:
                pieces.pop(0)[1]()
            fin_norm(NQT - 1)
            fin_out(NQT - 1, split_dma=True)

    return nc


def _rope_tables(token_positions):
    """cosP/sinPs in the transposed per-partition layout (bf16).

    Per-head row layout (64 rows, two 32-lane groups): group g holds rope
    pairs j = 16g..16g+15, with even dims on lanes 0-15 and odd dims on lanes
    16-31 of the group.  The rope partner is lane^16, reachable by
    stream_shuffle within a 32-lane group.  sinPs multiplies q BEFORE the
    swap: odd-dim rows carry -sin (their product lands on the even output),
    even-dim rows carry +sin.
    """
    pos = token_positions.astype(np.float32)  # [S]
    inv = (1.0 / (10000.0 ** (np.arange(0, HD, 2, dtype=np.float32) / HD)))
    freqs = pos[:, None] * inv[None, :]        # [S, 32]
    cos32 = np.cos(freqs).T.astype(np.float32)  # [32, S], row j = pair j
    sin32 = np.sin(freqs).T.astype(np.float32)
    cos64 = np.concatenate([cos32[0:16], cos32[0:16], cos32[16:32], cos32[16:32]], 0)
    sin64 = np.concatenate([sin32[0:16], -sin32[0:16], sin32[16:32], -sin32[16:32]], 0)
    cosP = np.concatenate([cos64, cos64], 0)
    sinPs = np.concatenate([sin64, sin64], 0)
    return (np.ascontiguousarray(cosP).astype(ml_dtypes.bfloat16),
            np.ascontiguousarray(sinPs).astype(ml_dtypes.bfloat16))


def kernel(x, w_q, w_k, w_v, w_o, token_positions):
    x = np.asarray(x, dtype=np.float32)
    w_q = np.asarray(w_q, dtype=np.float32)
    w_k = np.asarray(w_k, dtype=np.float32)
    w_v = np.asarray(w_v, dtype=np.float32)
    w_o = np.asarray(w_o, dtype=np.float32)
    tp = np.asarray(token_positions).reshape(-1)

    b = x.shape[0]
    assert x.shape == (b, S, D) and b == 1

    xT = np.ascontiguousarray(x[0].T).astype(ml_dtypes.bfloat16)  # [D, S]
    cosP, sinPs = _rope_tables(tp)

    # per-head permutation matching _rope_tables: per 32-lane group g, evens
    # of pairs 16g..16g+15 then odds of the same pairs
    j16 = np.arange(16)
    perm64 = np.concatenate([2 * j16, 2 * j16 + 1, 32 + 2 * j16, 32 + 2 * j16 + 1])

    if "nc" not in _CACHE:
        _CACHE["nc"] = _build_nc()
    nc = _CACHE["nc"]

    in_maps = []
    for c in range(NCORES):
        rows = np.concatenate(
            [c * 128 + h * 64 + perm64 for h in range(HPC)]
        )  # 128 permuted q/k output dims of this core
        wq_p = w_q[rows]                    # [128, D]
        wk_p = w_k[rows]
        in_maps.append({
            "xT": xT,
            "wq": np.ascontiguousarray(wq_p.T).astype(ml_dtypes.bfloat16),
            "wk": np.ascontiguousarray(wk_p.T).astype(ml_dtypes.bfloat16),
            "wv": np.ascontiguousarray(w_v[c * 128:(c + 1) * 128].T).astype(ml_dtypes.bfloat16),
            "wo": np.ascontiguousarray(w_o[:, c * 128:(c + 1) * 128].T).astype(ml_dtypes.bfloat16),
            "cosP": cosP,
            "sinPs": sinPs,
        })

    _CACHE["last_in_maps"] = in_maps
    res = run_bass_kernel_spmd(nc, in_maps, core_ids=list(range(NCORES)))
    out = np.zeros((S, D), np.float32)
    for c in range(NCORES):
        out += res.results[c]["out"].astype(np.float32)
    return out[None]


if __name__ == "__main__":
    rng = np.random.default_rng(0)
    x = rng.standard_normal((1, S, D), dtype=np.float32)
    sc = 1.0 / np.sqrt(D)
    wq = rng.standard_normal((D, D), dtype=np.float32) * sc
    wk = rng.standard_normal((D, D), dtype=np.float32) * sc
    wv = rng.standard_normal((D, D), dtype=np.float32) * sc
    wo = rng.standard_normal((D, D), dtype=np.float32) * sc
    tpos = np.arange(S, dtype=np.int32)[None]
    out = kernel(x=x, w_q=wq, w_k=wk, w_v=wv, w_o=wo, token_positions=tpos)
    print("kernel out:", out.shape, out.dtype, float(np.abs(out).max()))
